# revision 10
# baseline (speedup 1.0000x reference)
"""EUNN cell (B=2048, H=1024, capacity=128) on 8 NeuronCores.

The 128 Givens layers compose into a banded complex matrix M (bandwidth 128,
block-tridiagonal in 128-blocks); out = D_omega M x. Host composes M (f64) and
quantizes M and x into fp8e4m3 (value, residual) pairs; the device computes the
complex banded matvec with fp8 DoubleRow matmuls (2 K-rows per PE pass, 0.5
cycles/row), accumulating main + x-residual + M-residual terms in fp32 PSUM.
Dropped residual*residual cross terms leave ~9e-3 relative error (gate 2e-2).

Sharding: 8 cores = 4 batch quarters x 2 hidden halves. Per core: 11 real
(row-block, col-block) pairs, 66 DoubleRow matmuls into 8 PSUM banks, outputs
streamed per row-block as fp16.
"""
import numpy as np

H = 1024
B = 2048
CAP = 128
EH = H // 2
OH = (H - 1) // 2
EC = (CAP + 1) // 2
OC = CAP // 2
BAND = CAP
NC_CORES = 8
NB = H // 128          # 8 hidden blocks
NJ = 2                 # hidden halves
NI = 4                 # batch quarters
BCORE = B // NI        # 512 batch cols per core
RH = NB // NJ          # 4 r-blocks per core
CR = RH + 1            # 5 real c-blocks per core (1-block halo)
# (rl, c) pairs per core: rl=0 has 2 cols, rl>=1 have 3
PAIR_COLS = [[0, 1], [0, 1, 2], [1, 2, 3], [2, 3, 4]]
NPAIR = sum(len(c) for c in PAIR_COLS)  # 11
NCOMP = 6              # m comps per pair: [nMia, Mra, Mia, nMib, Mrb, Mib]
NSLAB = 4              # x slabs per c-block: [xia, xra, xib, xrb]

_perm_even = np.arange(EH * 2).reshape(-1, 2)[:, ::-1].reshape(-1)
_perm_odd = np.concatenate(
    [[0], np.arange(1, OH * 2 + 1).reshape(-1, 2)[:, ::-1].reshape(-1), [OH * 2 + 1]]
)


def _interleave(a, b):
    return np.stack([a, b], axis=-1).reshape(-1)


def _layer_coeffs(even_theta, odd_theta, even_phi, odd_phi):
    ce, se = np.cos(even_theta), np.sin(even_theta)
    cpe, spe = np.cos(even_phi), np.sin(even_phi)
    co, so = np.cos(odd_theta), np.sin(odd_theta)
    cpo, spo = np.cos(odd_phi), np.sin(odd_phi)
    zE = np.zeros(EH)
    zO = np.zeros(OH)
    one = np.ones(1)
    zero = np.zeros(1)
    for t in range(EC):
        ect, est, ecp, esp = ce[t], se[t], cpe[t], spe[t]
        v1 = _interleave(esp * ect, ect) + 1j * _interleave(ecp * ect, zE)
        v2 = _interleave(-esp * est, est) + 1j * _interleave(-ecp * est, zE)
        yield v1, v2, _perm_even
        oct_, ost, ocp, osp = co[t], so[t], cpo[t], spo[t]
        v1 = np.concatenate([one, _interleave(osp * oct_, oct_), one]) + 1j * np.concatenate(
            [zero, _interleave(ocp * oct_, zO), zero]
        )
        v2 = np.concatenate([zero, _interleave(-osp * ost, ost), zero]) + 1j * np.concatenate(
            [zero, _interleave(-ocp * ost, zO), zero]
        )
        yield v1, v2, _perm_odd


def _compose_banded(even_theta, odd_theta, even_phi, odd_phi):
    """M = L_128...L_1 as band array bnd[i, d], column j = i + d - BAND."""
    W = 2 * BAND + 1
    bnd = np.zeros((H, W), np.complex64)
    bnd[:, BAND] = 1.0
    new = np.zeros_like(bnd)
    for v1, v2, perm in _layer_coeffs(even_theta, odd_theta, even_phi, odd_phi):
        if perm is _perm_even:
            lo, hi = 0, H
        else:
            lo, hi = 1, H - 1
            new[0] = v1[0] * bnd[0]
            new[H - 1] = v1[H - 1] * bnd[H - 1]
        a = bnd[lo:hi:2]
        b = bnd[lo + 1:hi:2]
        v1a = v1[lo:hi:2, None]
        v2a = v2[lo:hi:2, None]
        v1b = v1[lo + 1:hi:2, None]
        v2b = v2[lo + 1:hi:2, None]
        na = new[lo:hi:2]
        nb = new[lo + 1:hi:2]
        np.multiply(v1a, a, out=na)
        na[:, 1:] += (v2a * b[:, :-1]).astype(np.complex64)
        np.multiply(v1b, b, out=nb)
        nb[:, :-1] += (v2b * a[:, 1:]).astype(np.complex64)
        bnd, new = new, bnd
    return bnd


def _banded_to_dense(bnd):
    M = np.zeros((H, H), bnd.dtype)
    rows = np.arange(H)
    for d in range(2 * BAND + 1):
        j = rows + d - BAND
        ok = (j >= 0) & (j < H)
        M[rows[ok], j[ok]] = bnd[ok, d]
    return M


_NC_CACHE = {}


def _build_device_kernel():
    if "nc" in _NC_CACHE:
        return _NC_CACHE["nc"]
    import concourse.tile as tile
    from concourse import bacc, mybir

    f8 = mybir.dt.float8e4
    f16 = mybir.dt.float16
    f32 = mybir.dt.float32
    DR = mybir.MatmulPerfMode.DoubleRow

    nc = bacc.Bacc("TRN2", target_bir_lowering=False, debug=False)
    x_d = nc.dram_tensor("x", [128, CR * NSLAB * BCORE], f8, kind="ExternalInput").ap()
    m_d = nc.dram_tensor("m", [128, NPAIR * NCOMP * 128], f8, kind="ExternalInput").ap()
    y_d = nc.dram_tensor("y", [128, RH * 2 * BCORE], f16, kind="ExternalOutput").ap()

    x_v = x_d.rearrange("p (c s b) -> p c s b", c=CR, s=NSLAB)
    m_v = m_d.rearrange("p (q n k) -> p q n k", q=NPAIR, n=NCOMP)
    y_v = y_d.rearrange("p (r s b) -> p r s b", r=RH, s=2)

    # m chunk per rl (pair ranges, rl0 split per pair), x chunk per c-block;
    # interleaved so the first psum group unblocks as early as possible. Every
    # DMA completion semaphore costs ~900ns to propagate, so chunks must land
    # well before their first consumer.
    m_ranges = []
    p0 = 0
    for cols in PAIR_COLS:
        m_ranges.append((p0, p0 + len(cols)))
        p0 += len(cols)

    with tile.TileContext(nc) as tc:
        with (
            tc.tile_pool(name="mp", bufs=1) as mpool,
            tc.tile_pool(name="xp", bufs=1) as xpool,
            tc.tile_pool(name="op", bufs=1) as opool,
            tc.tile_pool(name="pp", bufs=1, space="PSUM") as pspool,
        ):
            m_t = mpool.tile([128, NPAIR * NCOMP * 128], f8, tag="m")
            x_t = xpool.tile([128, CR * NSLAB * BCORE], f8, tag="x")
            y_t = opool.tile([128, RH * 2 * BCORE], f16, tag="y")
            m_r = m_t[:].rearrange("p (q n k) -> p q n k", q=NPAIR, n=NCOMP)
            x_r = x_t[:].rearrange("p (c s b) -> p c s b", c=CR, s=NSLAB)
            y_r = y_t[:].rearrange("p (r s b) -> p r s b", r=RH, s=2)

            # interleaved input DMA schedule: x0 first (largest early dep),
            # then m_rl0, then alternating chunks in consumption order.
            for kind, idx in (
                ("x", 0), ("m", 0), ("x", 1), ("m", 1),
                ("x", 2), ("m", 2), ("x", 3), ("m", 3), ("x", 4),
            ):
                if kind == "m":
                    a, b = m_ranges[idx]
                    nc.sync.dma_start(m_r[:, a:b], m_v[:, a:b])
                else:
                    nc.sync.dma_start(x_r[:, idx], x_v[:, idx])

            # 8 PSUM banks: rl0 and rl2 share a tile pair (rl0's copies are
            # long done before rl2 accumulates), rl1 has its own, and the last
            # r-block uses four half-width tiles so its two batch-half groups
            # carry no false tile-granular dependency.
            psA = [pspool.tile([128, BCORE], f32, tag=f"psA{s}", name=f"psA{s}")
                   for s in range(2)]
            psB = [pspool.tile([128, BCORE], f32, tag=f"psB{s}", name=f"psB{s}")
                   for s in range(2)]
            psr = [psA[0], psB[0], psA[0]]
            psi = [psA[1], psB[1], psA[1]]
            hb = BCORE // 2
            psrh = [pspool.tile([128, hb], f32, tag=f"psrh{h}", name=f"psrh{h}")
                    for h in range(2)]
            psih = [pspool.tile([128, hb], f32, tag=f"psih{h}", name=f"psih{h}")
                    for h in range(2)]

            def emit_group(rl, bsl, tr, ti):
                cols = PAIR_COLS[rl]
                a, _ = m_ranges[rl]
                n = len(cols)
                for k, c in enumerate(cols):
                    p = a + k
                    first = k == 0
                    last = k == n - 1
                    # rhs pairs: lo = (xia, xra), hi = (xib, xrb)
                    rlo = x_r[:, c, 0:2, bsl]
                    rhi = x_r[:, c, 2:4, bsl]
                    # lhsT pairs within comps [nMia, Mra, Mia, nMib, Mrb, Mib]
                    pr_m = m_r[:, p, 0:2]   # (nMia, Mra): psr main/xres
                    pi_m = m_r[:, p, 1:3]   # (Mra, Mia):  psi main/xres
                    pr_r = m_r[:, p, 3:5]   # (nMib, Mrb): psr Mres
                    pi_r = m_r[:, p, 4:6]   # (Mrb, Mib):  psi Mres
                    nc.tensor.matmul(tr, lhsT=pr_m, rhs=rlo,
                                     start=first, stop=False, perf_mode=DR)
                    nc.tensor.matmul(ti, lhsT=pi_m, rhs=rlo,
                                     start=first, stop=False, perf_mode=DR)
                    nc.tensor.matmul(tr, lhsT=pr_m, rhs=rhi,
                                     start=False, stop=False, perf_mode=DR)
                    nc.tensor.matmul(ti, lhsT=pi_m, rhs=rhi,
                                     start=False, stop=False, perf_mode=DR)
                    nc.tensor.matmul(tr, lhsT=pr_r, rhs=rlo,
                                     start=False, stop=last, perf_mode=DR)
                    nc.tensor.matmul(ti, lhsT=pi_r, rhs=rlo,
                                     start=False, stop=last, perf_mode=DR)
                # PSUM -> SBUF fp16, split across DVE and Act engines
                nc.vector.tensor_copy(y_r[:, rl, 0, bsl], tr)
                nc.scalar.copy(y_r[:, rl, 1, bsl], ti)

            full = slice(0, BCORE)
            for rl in range(RH - 1):
                emit_group(rl, full, psr[rl][:], psi[rl][:])
            # last r-block in two batch halves: the second half's copies and
            # output DMA are all that trail the final matmul.
            emit_group(RH - 1, slice(0, hb), psrh[0][:], psih[0][:])
            emit_group(RH - 1, slice(hb, BCORE), psrh[1][:], psih[1][:])

            for rl in range(RH - 1):
                nc.sync.dma_start(y_v[:, rl], y_r[:, rl])
            nc.sync.dma_start(y_v[:, RH - 1, :, 0:hb], y_r[:, RH - 1, :, 0:hb])
            nc.sync.dma_start(y_v[:, RH - 1, :, hb:BCORE],
                              y_r[:, RH - 1, :, hb:BCORE])

    nc.compile()
    _NC_CACHE["nc"] = nc
    return nc


def _host_prepare(x_re, x_im, omega, even_theta, odd_theta, even_phi, odd_phi):
    """Compose M, fold omega, quantize to fp8 (value, residual) pairs, pack."""
    import ml_dtypes

    F8 = ml_dtypes.float8_e4m3

    def q8(a):
        return np.asarray(a, np.float32).astype(F8)

    bnd = _compose_banded(
        even_theta.astype(np.float64),
        odd_theta.astype(np.float64),
        even_phi.astype(np.float64),
        odd_phi.astype(np.float64),
    )
    M = _banded_to_dense(bnd)
    w = omega.astype(np.float64)
    Mw = (np.cos(w) + 1j * np.sin(w))[:, None] * M
    Mre = np.asarray(Mw.real, np.float32)
    Mim = np.asarray(Mw.imag, np.float32)
    Mra = q8(Mre)
    Mrb = q8(Mre - Mra.astype(np.float32))
    Mia = q8(Mim)
    Mib = q8(Mim - Mia.astype(np.float32))

    xrT = np.ascontiguousarray(x_re.T).astype(np.float32)  # [H, B]
    xiT = np.ascontiguousarray(x_im.T).astype(np.float32)
    XRA = q8(xrT)
    XRB = q8(xrT - XRA.astype(np.float32))
    XIA = q8(xiT)
    XIB = q8(xiT - XIA.astype(np.float32))

    # m packs per hidden half (shared by the 4 batch quarters)
    m_packs = []
    for j in range(NJ):
        m_p = np.zeros((128, NPAIR, NCOMP, 128), F8)
        p = 0
        for rl in range(RH):
            r = rl if j == 0 else NB - 1 - rl
            for cl in PAIR_COLS[rl]:
                c = cl if j == 0 else NB - 1 - cl
                rs = slice(r * 128, (r + 1) * 128)
                cs = slice(c * 128, (c + 1) * 128)
                # lhsT: [K = c rows, out = r cols]
                m_p[:, p, 1] = Mra[rs, cs].T
                m_p[:, p, 2] = Mia[rs, cs].T
                m_p[:, p, 4] = Mrb[rs, cs].T
                m_p[:, p, 5] = Mib[rs, cs].T
                m_p[:, p, 0] = -m_p[:, p, 2]  # nMia
                m_p[:, p, 3] = -m_p[:, p, 5]  # nMib
                p += 1
        m_packs.append(np.ascontiguousarray(m_p.reshape(128, -1)))

    in_maps = []
    for core in range(NC_CORES):
        j, i = divmod(core, NI)
        bs = slice(i * BCORE, (i + 1) * BCORE)
        x_s = np.empty((128, CR, NSLAB, BCORE), F8)
        for s in range(CR):
            g = s if j == 0 else NB - 1 - s
            gs = slice(g * 128, (g + 1) * 128)
            x_s[:, s, 0] = XIA[gs, bs]
            x_s[:, s, 1] = XRA[gs, bs]
            x_s[:, s, 2] = XIB[gs, bs]
            x_s[:, s, 3] = XRB[gs, bs]
        in_maps.append({"x": np.ascontiguousarray(x_s.reshape(128, -1)),
                        "m": m_packs[j]})
    return in_maps


def kernel(x_re, x_im, omega, even_theta, odd_theta, even_phi, odd_phi):
    from concourse.bass_utils import run_bass_kernel_spmd

    in_maps = _host_prepare(
        np.asarray(x_re, np.float32),
        np.asarray(x_im, np.float32),
        np.asarray(omega),
        np.asarray(even_theta),
        np.asarray(odd_theta),
        np.asarray(even_phi),
        np.asarray(odd_phi),
    )
    nc = _build_device_kernel()
    res = run_bass_kernel_spmd(nc, in_maps, core_ids=list(range(NC_CORES)))
    yreT = np.empty((H, B), np.float32)
    yimT = np.empty((H, B), np.float32)
    for core in range(NC_CORES):
        j, i = divmod(core, NI)
        bs = slice(i * BCORE, (i + 1) * BCORE)
        y = res.results[core]["y"].reshape(128, RH, 2, BCORE)
        for rl in range(RH):
            r = rl if j == 0 else NB - 1 - rl
            rs = slice(r * 128, (r + 1) * 128)
            yreT[rs, bs] = y[:, rl, 0].astype(np.float32)
            yimT[rs, bs] = y[:, rl, 1].astype(np.float32)
    out_re = np.ascontiguousarray(yreT.T)
    out_im = np.ascontiguousarray(yimT.T)
    return out_re, out_im


# revision 32
# speedup vs baseline: 1.1966x; 1.1966x over previous
"""EUNN cell (B=2048, H=1024, capacity=128) on 8 NeuronCores.

The 128 Givens layers compose into a banded complex matrix M (bandwidth 128,
block-tridiagonal in 128-blocks); out = D_omega M x. Host composes M (f64) and
quantizes M and x into fp8e4m3 (value, residual) pairs; the device computes the
complex banded matvec with fp8 DoubleRow matmuls (2 K-rows per PE pass, 0.5
cycles/row), accumulating main + x-residual + M-residual terms in fp32 PSUM.
Dropped residual*residual cross terms leave ~9e-3 relative error (gate 2e-2).

Sharding: 8 cores = 4 batch quarters x 2 hidden halves. Per core: 11 real
(row-block, col-block) pairs, 66 DoubleRow matmuls into 8 PSUM banks, outputs
streamed per row-block as fp16.
"""
import numpy as np

H = 1024
B = 2048
CAP = 128
EH = H // 2
OH = (H - 1) // 2
EC = (CAP + 1) // 2
OC = CAP // 2
BAND = CAP
NC_CORES = 8
NB = H // 128          # 8 hidden blocks
NJ = 2                 # hidden halves
NI = 4                 # batch quarters
BCORE = B // NI        # 512 batch cols per core
RH = NB // NJ          # 4 r-blocks per core
CR = RH + 1            # 5 real c-blocks per core (1-block halo)
# (rl, c) pairs per core: rl=0 has 2 cols, rl>=1 have 3
PAIR_COLS = [[0, 1], [0, 1, 2], [1, 2, 3], [2, 3, 4]]
NPAIR = sum(len(c) for c in PAIR_COLS)  # 11
NCOMP = 6              # m comps per pair: [nMia, Mra, Mia, nMib, Mrb, Mib]
NSLAB = 4              # x slabs per c-block: [xia, xra, xib, xrb]

_perm_even = np.arange(EH * 2).reshape(-1, 2)[:, ::-1].reshape(-1)
_perm_odd = np.concatenate(
    [[0], np.arange(1, OH * 2 + 1).reshape(-1, 2)[:, ::-1].reshape(-1), [OH * 2 + 1]]
)


def _interleave(a, b):
    return np.stack([a, b], axis=-1).reshape(-1)


def _layer_coeffs(even_theta, odd_theta, even_phi, odd_phi):
    ce, se = np.cos(even_theta), np.sin(even_theta)
    cpe, spe = np.cos(even_phi), np.sin(even_phi)
    co, so = np.cos(odd_theta), np.sin(odd_theta)
    cpo, spo = np.cos(odd_phi), np.sin(odd_phi)
    zE = np.zeros(EH)
    zO = np.zeros(OH)
    one = np.ones(1)
    zero = np.zeros(1)
    for t in range(EC):
        ect, est, ecp, esp = ce[t], se[t], cpe[t], spe[t]
        v1 = _interleave(esp * ect, ect) + 1j * _interleave(ecp * ect, zE)
        v2 = _interleave(-esp * est, est) + 1j * _interleave(-ecp * est, zE)
        yield v1, v2, _perm_even
        oct_, ost, ocp, osp = co[t], so[t], cpo[t], spo[t]
        v1 = np.concatenate([one, _interleave(osp * oct_, oct_), one]) + 1j * np.concatenate(
            [zero, _interleave(ocp * oct_, zO), zero]
        )
        v2 = np.concatenate([zero, _interleave(-osp * ost, ost), zero]) + 1j * np.concatenate(
            [zero, _interleave(-ocp * ost, zO), zero]
        )
        yield v1, v2, _perm_odd


def _compose_banded(even_theta, odd_theta, even_phi, odd_phi):
    """M = L_128...L_1 as band array bnd[i, d], column j = i + d - BAND."""
    W = 2 * BAND + 1
    bnd = np.zeros((H, W), np.complex64)
    bnd[:, BAND] = 1.0
    new = np.zeros_like(bnd)
    for v1, v2, perm in _layer_coeffs(even_theta, odd_theta, even_phi, odd_phi):
        if perm is _perm_even:
            lo, hi = 0, H
        else:
            lo, hi = 1, H - 1
            new[0] = v1[0] * bnd[0]
            new[H - 1] = v1[H - 1] * bnd[H - 1]
        a = bnd[lo:hi:2]
        b = bnd[lo + 1:hi:2]
        v1a = v1[lo:hi:2, None]
        v2a = v2[lo:hi:2, None]
        v1b = v1[lo + 1:hi:2, None]
        v2b = v2[lo + 1:hi:2, None]
        na = new[lo:hi:2]
        nb = new[lo + 1:hi:2]
        np.multiply(v1a, a, out=na)
        na[:, 1:] += (v2a * b[:, :-1]).astype(np.complex64)
        np.multiply(v1b, b, out=nb)
        nb[:, :-1] += (v2b * a[:, 1:]).astype(np.complex64)
        bnd, new = new, bnd
    return bnd


def _banded_to_dense(bnd):
    M = np.zeros((H, H), bnd.dtype)
    rows = np.arange(H)
    for d in range(2 * BAND + 1):
        j = rows + d - BAND
        ok = (j >= 0) & (j < H)
        M[rows[ok], j[ok]] = bnd[ok, d]
    return M


_NC_CACHE = {}


def _build_device_kernel():
    if "nc" in _NC_CACHE:
        return _NC_CACHE["nc"]
    import concourse.tile as tile
    from concourse import bacc, mybir

    f8 = mybir.dt.float8e4
    f16 = mybir.dt.float16
    f32 = mybir.dt.float32
    DR = mybir.MatmulPerfMode.DoubleRow

    nc = bacc.Bacc("TRN2", target_bir_lowering=False, debug=False)
    x_d = nc.dram_tensor("x", [128, CR * NSLAB * BCORE], f8, kind="ExternalInput").ap()
    m_d = nc.dram_tensor("m", [128, NPAIR * NCOMP * 128], f8, kind="ExternalInput").ap()
    y_d = nc.dram_tensor("y", [128, RH * 2 * BCORE], f16, kind="ExternalOutput").ap()

    x_v = x_d.rearrange("p (c s b) -> p c s b", c=CR, s=NSLAB)
    m_v = m_d.rearrange("p (q n k) -> p q n k", q=NPAIR, n=NCOMP)
    y_v = y_d.rearrange("p (r s b) -> p r s b", r=RH, s=2)

    # m chunk per rl (pair ranges, rl0 split per pair), x chunk per c-block;
    # interleaved so the first psum group unblocks as early as possible. Every
    # DMA completion semaphore costs ~900ns to propagate, so chunks must land
    # well before their first consumer.
    m_ranges = []
    p0 = 0
    for cols in PAIR_COLS:
        m_ranges.append((p0, p0 + len(cols)))
        p0 += len(cols)

    with tile.TileContext(nc) as tc:
        with (
            tc.tile_pool(name="mp", bufs=1) as mpool,
            tc.tile_pool(name="xp", bufs=1) as xpool,
            tc.tile_pool(name="op", bufs=1) as opool,
            tc.tile_pool(name="pp", bufs=1, space="PSUM") as pspool,
        ):
            m_t = mpool.tile([128, NPAIR * NCOMP * 128], f8, tag="m")
            x_t = xpool.tile([128, CR * NSLAB * BCORE], f8, tag="x")
            y_t = opool.tile([128, RH * 2 * BCORE], f16, tag="y")
            m_r = m_t[:].rearrange("p (q n k) -> p q n k", q=NPAIR, n=NCOMP)
            x_r = x_t[:].rearrange("p (c s b) -> p c s b", c=CR, s=NSLAB)
            y_r = y_t[:].rearrange("p (r s b) -> p r s b", r=RH, s=2)

            # interleaved input DMA schedule in consumption order. NOTE: the
            # m chunk must be issued first -- leading with an x chunk flips
            # the PE into a low p-state regime and costs ~4us.
            for kind, idx in (
                ("m", 0), ("x", 0), ("x", 1), ("m", 1),
                ("x", 2), ("m", 2), ("x", 3), ("m", 3), ("x", 4),
            ):
                if kind == "m":
                    a, b = m_ranges[idx]
                    nc.sync.dma_start(m_r[:, a:b], m_v[:, a:b])
                else:
                    nc.sync.dma_start(x_r[:, idx], x_v[:, idx])

            psr = [pspool.tile([128, BCORE], f32, tag=f"psr{r}", name=f"psr{r}")
                   for r in range(RH)]
            psi = [pspool.tile([128, BCORE], f32, tag=f"psi{r}", name=f"psi{r}")
                   for r in range(RH)]

            def emit_group(rl, bsl, tr, ti):
                cols = PAIR_COLS[rl]
                a, _ = m_ranges[rl]
                n = len(cols)
                for k, c in enumerate(cols):
                    p = a + k
                    first = k == 0
                    last = k == n - 1
                    # rhs pairs: lo = (xia, xra), hi = (xib, xrb)
                    rlo = x_r[:, c, 0:2, bsl]
                    rhi = x_r[:, c, 2:4, bsl]
                    # lhsT pairs within comps [nMia, Mra, Mia, nMib, Mrb, Mib]
                    pr_m = m_r[:, p, 0:2]   # (nMia, Mra): psr main/xres
                    pi_m = m_r[:, p, 1:3]   # (Mra, Mia):  psi main/xres
                    pr_r = m_r[:, p, 3:5]   # (nMib, Mrb): psr Mres
                    pi_r = m_r[:, p, 4:6]   # (Mrb, Mib):  psi Mres
                    nc.tensor.matmul(tr, lhsT=pr_m, rhs=rlo,
                                     start=first, stop=False, perf_mode=DR)
                    nc.tensor.matmul(ti, lhsT=pi_m, rhs=rlo,
                                     start=first, stop=False, perf_mode=DR)
                    nc.tensor.matmul(tr, lhsT=pr_m, rhs=rhi,
                                     start=False, stop=False, perf_mode=DR)
                    nc.tensor.matmul(ti, lhsT=pi_m, rhs=rhi,
                                     start=False, stop=False, perf_mode=DR)
                    nc.tensor.matmul(tr, lhsT=pr_r, rhs=rlo,
                                     start=False, stop=last, perf_mode=DR)
                    nc.tensor.matmul(ti, lhsT=pi_r, rhs=rlo,
                                     start=False, stop=last, perf_mode=DR)
                # PSUM -> SBUF fp16, split across DVE and Act engines
                nc.vector.tensor_copy(y_r[:, rl, 0, bsl], tr)
                nc.scalar.copy(y_r[:, rl, 1, bsl], ti)

            full = slice(0, BCORE)
            for rl in range(RH):
                emit_group(rl, full, psr[rl][:], psi[rl][:])

            for rl in range(RH):
                nc.sync.dma_start(y_v[:, rl], y_r[:, rl])

    nc.compile()
    _NC_CACHE["nc"] = nc
    return nc


def _host_prepare(x_re, x_im, omega, even_theta, odd_theta, even_phi, odd_phi):
    """Compose M, fold omega, quantize to fp8 (value, residual) pairs, pack."""
    import ml_dtypes

    F8 = ml_dtypes.float8_e4m3

    def q8(a):
        return np.asarray(a, np.float32).astype(F8)

    bnd = _compose_banded(
        even_theta.astype(np.float64),
        odd_theta.astype(np.float64),
        even_phi.astype(np.float64),
        odd_phi.astype(np.float64),
    )
    M = _banded_to_dense(bnd)
    w = omega.astype(np.float64)
    Mw = (np.cos(w) + 1j * np.sin(w))[:, None] * M
    Mre = np.asarray(Mw.real, np.float32)
    Mim = np.asarray(Mw.imag, np.float32)
    Mra = q8(Mre)
    Mrb = q8(Mre - Mra.astype(np.float32))
    Mia = q8(Mim)
    Mib = q8(Mim - Mia.astype(np.float32))

    xrT = np.ascontiguousarray(x_re.T).astype(np.float32)  # [H, B]
    xiT = np.ascontiguousarray(x_im.T).astype(np.float32)
    XRA = q8(xrT)
    XRB = q8(xrT - XRA.astype(np.float32))
    XIA = q8(xiT)
    XIB = q8(xiT - XIA.astype(np.float32))

    # m packs per hidden half (shared by the 4 batch quarters)
    m_packs = []
    for j in range(NJ):
        m_p = np.zeros((128, NPAIR, NCOMP, 128), F8)
        p = 0
        for rl in range(RH):
            r = rl if j == 0 else NB - 1 - rl
            for cl in PAIR_COLS[rl]:
                c = cl if j == 0 else NB - 1 - cl
                rs = slice(r * 128, (r + 1) * 128)
                cs = slice(c * 128, (c + 1) * 128)
                # lhsT: [K = c rows, out = r cols]
                m_p[:, p, 1] = Mra[rs, cs].T
                m_p[:, p, 2] = Mia[rs, cs].T
                m_p[:, p, 4] = Mrb[rs, cs].T
                m_p[:, p, 5] = Mib[rs, cs].T
                m_p[:, p, 0] = -m_p[:, p, 2]  # nMia
                m_p[:, p, 3] = -m_p[:, p, 5]  # nMib
                p += 1
        m_packs.append(np.ascontiguousarray(m_p.reshape(128, -1)))

    in_maps = []
    for core in range(NC_CORES):
        j, i = divmod(core, NI)
        bs = slice(i * BCORE, (i + 1) * BCORE)
        x_s = np.empty((128, CR, NSLAB, BCORE), F8)
        for s in range(CR):
            g = s if j == 0 else NB - 1 - s
            gs = slice(g * 128, (g + 1) * 128)
            x_s[:, s, 0] = XIA[gs, bs]
            x_s[:, s, 1] = XRA[gs, bs]
            x_s[:, s, 2] = XIB[gs, bs]
            x_s[:, s, 3] = XRB[gs, bs]
        in_maps.append({"x": np.ascontiguousarray(x_s.reshape(128, -1)),
                        "m": m_packs[j]})
    return in_maps


def kernel(x_re, x_im, omega, even_theta, odd_theta, even_phi, odd_phi):
    from concourse.bass_utils import run_bass_kernel_spmd

    in_maps = _host_prepare(
        np.asarray(x_re, np.float32),
        np.asarray(x_im, np.float32),
        np.asarray(omega),
        np.asarray(even_theta),
        np.asarray(odd_theta),
        np.asarray(even_phi),
        np.asarray(odd_phi),
    )
    nc = _build_device_kernel()
    res = run_bass_kernel_spmd(nc, in_maps, core_ids=list(range(NC_CORES)))
    yreT = np.empty((H, B), np.float32)
    yimT = np.empty((H, B), np.float32)
    for core in range(NC_CORES):
        j, i = divmod(core, NI)
        bs = slice(i * BCORE, (i + 1) * BCORE)
        y = res.results[core]["y"].reshape(128, RH, 2, BCORE)
        for rl in range(RH):
            r = rl if j == 0 else NB - 1 - rl
            rs = slice(r * 128, (r + 1) * 128)
            yreT[rs, bs] = y[:, rl, 0].astype(np.float32)
            yimT[rs, bs] = y[:, rl, 1].astype(np.float32)
    out_re = np.ascontiguousarray(yreT.T)
    out_im = np.ascontiguousarray(yimT.T)
    return out_re, out_im


# revision 38
# speedup vs baseline: 1.2193x; 1.0190x over previous
"""EUNN cell (B=2048, H=1024, capacity=128) on 8 NeuronCores.

The 128 Givens layers compose into a banded complex matrix M (bandwidth 128,
block-tridiagonal in 128-blocks); out = D_omega M x. Host composes M (f64) and
quantizes M and x into fp8e4m3 (value, residual) pairs; the device computes the
complex banded matvec with fp8 DoubleRow matmuls (2 K-rows per PE pass, 0.5
cycles/row), accumulating main + x-residual + M-residual terms in fp32 PSUM.
Dropped residual*residual cross terms leave ~9e-3 relative error (gate 2e-2).

Sharding: 8 cores = 4 batch quarters x 2 hidden halves. Per core: 11 real
(row-block, col-block) pairs, 66 DoubleRow matmuls into 8 PSUM banks, outputs
streamed per row-block as fp16.
"""
import numpy as np

H = 1024
B = 2048
CAP = 128
EH = H // 2
OH = (H - 1) // 2
EC = (CAP + 1) // 2
OC = CAP // 2
BAND = CAP
NC_CORES = 8
NB = H // 128          # 8 hidden blocks
NJ = 2                 # hidden halves
NI = 4                 # batch quarters
BCORE = B // NI        # 512 batch cols per core
RH = NB // NJ          # 4 r-blocks per core
CR = RH + 1            # 5 real c-blocks per core (1-block halo)
# (rl, c) pairs per core: rl=0 has 2 cols, rl>=1 have 3
PAIR_COLS = [[0, 1], [0, 1, 2], [1, 2, 3], [2, 3, 4]]
NPAIR = sum(len(c) for c in PAIR_COLS)  # 11
NCOMP = 6              # m comps per pair: [nMia, Mra, Mia, nMib, Mrb, Mib]
NSLAB = 4              # x slabs per c-block: [xia, xra, xib, xrb]

_perm_even = np.arange(EH * 2).reshape(-1, 2)[:, ::-1].reshape(-1)
_perm_odd = np.concatenate(
    [[0], np.arange(1, OH * 2 + 1).reshape(-1, 2)[:, ::-1].reshape(-1), [OH * 2 + 1]]
)


def _interleave(a, b):
    return np.stack([a, b], axis=-1).reshape(-1)


def _layer_coeffs(even_theta, odd_theta, even_phi, odd_phi):
    ce, se = np.cos(even_theta), np.sin(even_theta)
    cpe, spe = np.cos(even_phi), np.sin(even_phi)
    co, so = np.cos(odd_theta), np.sin(odd_theta)
    cpo, spo = np.cos(odd_phi), np.sin(odd_phi)
    zE = np.zeros(EH)
    zO = np.zeros(OH)
    one = np.ones(1)
    zero = np.zeros(1)
    for t in range(EC):
        ect, est, ecp, esp = ce[t], se[t], cpe[t], spe[t]
        v1 = _interleave(esp * ect, ect) + 1j * _interleave(ecp * ect, zE)
        v2 = _interleave(-esp * est, est) + 1j * _interleave(-ecp * est, zE)
        yield v1, v2, _perm_even
        oct_, ost, ocp, osp = co[t], so[t], cpo[t], spo[t]
        v1 = np.concatenate([one, _interleave(osp * oct_, oct_), one]) + 1j * np.concatenate(
            [zero, _interleave(ocp * oct_, zO), zero]
        )
        v2 = np.concatenate([zero, _interleave(-osp * ost, ost), zero]) + 1j * np.concatenate(
            [zero, _interleave(-ocp * ost, zO), zero]
        )
        yield v1, v2, _perm_odd


def _compose_banded(even_theta, odd_theta, even_phi, odd_phi):
    """M = L_128...L_1 as band array bnd[i, d], column j = i + d - BAND."""
    W = 2 * BAND + 1
    bnd = np.zeros((H, W), np.complex64)
    bnd[:, BAND] = 1.0
    new = np.zeros_like(bnd)
    for v1, v2, perm in _layer_coeffs(even_theta, odd_theta, even_phi, odd_phi):
        if perm is _perm_even:
            lo, hi = 0, H
        else:
            lo, hi = 1, H - 1
            new[0] = v1[0] * bnd[0]
            new[H - 1] = v1[H - 1] * bnd[H - 1]
        a = bnd[lo:hi:2]
        b = bnd[lo + 1:hi:2]
        v1a = v1[lo:hi:2, None]
        v2a = v2[lo:hi:2, None]
        v1b = v1[lo + 1:hi:2, None]
        v2b = v2[lo + 1:hi:2, None]
        na = new[lo:hi:2]
        nb = new[lo + 1:hi:2]
        np.multiply(v1a, a, out=na)
        na[:, 1:] += (v2a * b[:, :-1]).astype(np.complex64)
        np.multiply(v1b, b, out=nb)
        nb[:, :-1] += (v2b * a[:, 1:]).astype(np.complex64)
        bnd, new = new, bnd
    return bnd


def _banded_to_dense(bnd):
    M = np.zeros((H, H), bnd.dtype)
    rows = np.arange(H)
    for d in range(2 * BAND + 1):
        j = rows + d - BAND
        ok = (j >= 0) & (j < H)
        M[rows[ok], j[ok]] = bnd[ok, d]
    return M


_NC_CACHE = {}


def _build_device_kernel():
    if "nc" in _NC_CACHE:
        return _NC_CACHE["nc"]
    import concourse.tile as tile
    from concourse import bacc, mybir

    f8 = mybir.dt.float8e4
    f16 = mybir.dt.float16
    f32 = mybir.dt.float32
    DR = mybir.MatmulPerfMode.DoubleRow

    nc = bacc.Bacc("TRN2", target_bir_lowering=False, debug=False)
    x_d = nc.dram_tensor("x", [128, CR * NSLAB * BCORE], f8, kind="ExternalInput").ap()
    m_d = nc.dram_tensor("m", [128, NPAIR * NCOMP * 128], f8, kind="ExternalInput").ap()
    y_d = nc.dram_tensor("y", [128, RH * 2 * BCORE], f16, kind="ExternalOutput").ap()

    x_v = x_d.rearrange("p (c s b) -> p c s b", c=CR, s=NSLAB)
    m_v = m_d.rearrange("p (q n k) -> p q n k", q=NPAIR, n=NCOMP)
    y_v = y_d.rearrange("p (r s b) -> p r s b", r=RH, s=2)

    # m chunk per rl (pair ranges, rl0 split per pair), x chunk per c-block;
    # interleaved so the first psum group unblocks as early as possible. Every
    # DMA completion semaphore costs ~900ns to propagate, so chunks must land
    # well before their first consumer.
    m_ranges = []
    p0 = 0
    for cols in PAIR_COLS:
        m_ranges.append((p0, p0 + len(cols)))
        p0 += len(cols)

    with tile.TileContext(nc) as tc:
        with (
            tc.tile_pool(name="mp", bufs=1) as mpool,
            tc.tile_pool(name="xp", bufs=1) as xpool,
            tc.tile_pool(name="op", bufs=1) as opool,
            tc.tile_pool(name="pp", bufs=1, space="PSUM") as pspool,
        ):
            m_t = mpool.tile([128, NPAIR * NCOMP * 128], f8, tag="m")
            x_t = xpool.tile([128, CR * NSLAB * BCORE], f8, tag="x")
            y_t = opool.tile([128, RH * 2 * BCORE], f16, tag="y")
            m_r = m_t[:].rearrange("p (q n k) -> p q n k", q=NPAIR, n=NCOMP)
            x_r = x_t[:].rearrange("p (c s b) -> p c s b", c=CR, s=NSLAB)
            y_r = y_t[:].rearrange("p (r s b) -> p r s b", r=RH, s=2)

            # interleaved input DMA schedule in consumption order. NOTE: the
            # m chunk must be issued first -- leading with an x chunk flips
            # the PE into a low p-state regime and costs ~4us. m1 ships as
            # (first pair, rest) so rl1 can start the moment rl0 finishes.
            for kind, a, b in (
                ("m", 0, 2), ("x", 0, 0), ("x", 1, 1), ("m", 2, 3),
                ("m", 3, 5), ("x", 2, 2), ("m", 5, 6), ("m", 6, 8),
                ("x", 3, 3), ("m", 8, 9), ("m", 9, 11), ("x", 4, 4),
            ):
                if kind == "m":
                    nc.sync.dma_start(m_r[:, a:b], m_v[:, a:b])
                else:
                    nc.sync.dma_start(x_r[:, a], x_v[:, a])

            psr = [pspool.tile([128, BCORE], f32, tag=f"psr{r}", name=f"psr{r}")
                   for r in range(RH)]
            psi = [pspool.tile([128, BCORE], f32, tag=f"psi{r}", name=f"psi{r}")
                   for r in range(RH)]

            # tiny warmup matmul: anchors the PE p-state ramp clock early so
            # the first real matmuls run at full rate once their data lands
            nc.tensor.matmul(psr[0][:, 0:8], lhsT=m_r[:, 0, 0],
                             rhs=x_r[:, 0, 0, 0:8], start=True, stop=True)

            def emit_group(rl, bsl, tr, ti):
                cols = PAIR_COLS[rl]
                a, _ = m_ranges[rl]
                n = len(cols)
                for k, c in enumerate(cols):
                    p = a + k
                    first = k == 0
                    last = k == n - 1
                    # rhs pairs: lo = (xia, xra), hi = (xib, xrb)
                    rlo = x_r[:, c, 0:2, bsl]
                    rhi = x_r[:, c, 2:4, bsl]
                    # lhsT pairs within comps [nMia, Mra, Mia, nMib, Mrb, Mib]
                    pr_m = m_r[:, p, 0:2]   # (nMia, Mra): psr main/xres
                    pi_m = m_r[:, p, 1:3]   # (Mra, Mia):  psi main/xres
                    pr_r = m_r[:, p, 3:5]   # (nMib, Mrb): psr Mres
                    pi_r = m_r[:, p, 4:6]   # (Mrb, Mib):  psi Mres
                    nc.tensor.matmul(tr, lhsT=pr_m, rhs=rlo,
                                     start=first, stop=False, perf_mode=DR)
                    nc.tensor.matmul(ti, lhsT=pi_m, rhs=rlo,
                                     start=first, stop=False, perf_mode=DR)
                    nc.tensor.matmul(tr, lhsT=pr_m, rhs=rhi,
                                     start=False, stop=False, perf_mode=DR)
                    nc.tensor.matmul(ti, lhsT=pi_m, rhs=rhi,
                                     start=False, stop=False, perf_mode=DR)
                    nc.tensor.matmul(tr, lhsT=pr_r, rhs=rlo,
                                     start=False, stop=last, perf_mode=DR)
                    nc.tensor.matmul(ti, lhsT=pi_r, rhs=rlo,
                                     start=False, stop=last, perf_mode=DR)
                # PSUM -> SBUF fp16, split across DVE and Act engines
                nc.vector.tensor_copy(y_r[:, rl, 0, bsl], tr)
                nc.scalar.copy(y_r[:, rl, 1, bsl], ti)

            full = slice(0, BCORE)
            for rl in range(RH):
                emit_group(rl, full, psr[rl][:], psi[rl][:])

            for rl in range(RH):
                nc.sync.dma_start(y_v[:, rl], y_r[:, rl])

    nc.compile()
    _NC_CACHE["nc"] = nc
    return nc


def _host_prepare(x_re, x_im, omega, even_theta, odd_theta, even_phi, odd_phi):
    """Compose M, fold omega, quantize to fp8 (value, residual) pairs, pack."""
    import ml_dtypes

    F8 = ml_dtypes.float8_e4m3

    def q8(a):
        return np.asarray(a, np.float32).astype(F8)

    bnd = _compose_banded(
        even_theta.astype(np.float64),
        odd_theta.astype(np.float64),
        even_phi.astype(np.float64),
        odd_phi.astype(np.float64),
    )
    M = _banded_to_dense(bnd)
    w = omega.astype(np.float64)
    Mw = (np.cos(w) + 1j * np.sin(w))[:, None] * M
    Mre = np.asarray(Mw.real, np.float32)
    Mim = np.asarray(Mw.imag, np.float32)
    Mra = q8(Mre)
    Mrb = q8(Mre - Mra.astype(np.float32))
    Mia = q8(Mim)
    Mib = q8(Mim - Mia.astype(np.float32))

    xrT = np.ascontiguousarray(x_re.T).astype(np.float32)  # [H, B]
    xiT = np.ascontiguousarray(x_im.T).astype(np.float32)
    XRA = q8(xrT)
    XRB = q8(xrT - XRA.astype(np.float32))
    XIA = q8(xiT)
    XIB = q8(xiT - XIA.astype(np.float32))

    # m packs per hidden half (shared by the 4 batch quarters)
    m_packs = []
    for j in range(NJ):
        m_p = np.zeros((128, NPAIR, NCOMP, 128), F8)
        p = 0
        for rl in range(RH):
            r = rl if j == 0 else NB - 1 - rl
            for cl in PAIR_COLS[rl]:
                c = cl if j == 0 else NB - 1 - cl
                rs = slice(r * 128, (r + 1) * 128)
                cs = slice(c * 128, (c + 1) * 128)
                # lhsT: [K = c rows, out = r cols]
                m_p[:, p, 1] = Mra[rs, cs].T
                m_p[:, p, 2] = Mia[rs, cs].T
                m_p[:, p, 4] = Mrb[rs, cs].T
                m_p[:, p, 5] = Mib[rs, cs].T
                m_p[:, p, 0] = -m_p[:, p, 2]  # nMia
                m_p[:, p, 3] = -m_p[:, p, 5]  # nMib
                p += 1
        m_packs.append(np.ascontiguousarray(m_p.reshape(128, -1)))

    in_maps = []
    for core in range(NC_CORES):
        j, i = divmod(core, NI)
        bs = slice(i * BCORE, (i + 1) * BCORE)
        x_s = np.empty((128, CR, NSLAB, BCORE), F8)
        for s in range(CR):
            g = s if j == 0 else NB - 1 - s
            gs = slice(g * 128, (g + 1) * 128)
            x_s[:, s, 0] = XIA[gs, bs]
            x_s[:, s, 1] = XRA[gs, bs]
            x_s[:, s, 2] = XIB[gs, bs]
            x_s[:, s, 3] = XRB[gs, bs]
        in_maps.append({"x": np.ascontiguousarray(x_s.reshape(128, -1)),
                        "m": m_packs[j]})
    return in_maps


def kernel(x_re, x_im, omega, even_theta, odd_theta, even_phi, odd_phi):
    from concourse.bass_utils import run_bass_kernel_spmd

    in_maps = _host_prepare(
        np.asarray(x_re, np.float32),
        np.asarray(x_im, np.float32),
        np.asarray(omega),
        np.asarray(even_theta),
        np.asarray(odd_theta),
        np.asarray(even_phi),
        np.asarray(odd_phi),
    )
    nc = _build_device_kernel()
    res = run_bass_kernel_spmd(nc, in_maps, core_ids=list(range(NC_CORES)))
    yreT = np.empty((H, B), np.float32)
    yimT = np.empty((H, B), np.float32)
    for core in range(NC_CORES):
        j, i = divmod(core, NI)
        bs = slice(i * BCORE, (i + 1) * BCORE)
        y = res.results[core]["y"].reshape(128, RH, 2, BCORE)
        for rl in range(RH):
            r = rl if j == 0 else NB - 1 - rl
            rs = slice(r * 128, (r + 1) * 128)
            yreT[rs, bs] = y[:, rl, 0].astype(np.float32)
            yimT[rs, bs] = y[:, rl, 1].astype(np.float32)
    out_re = np.ascontiguousarray(yreT.T)
    out_im = np.ascontiguousarray(yimT.T)
    return out_re, out_im
